# revision 5
# baseline (speedup 1.0000x reference)
"""Relational GCN layer (gnn_message_passing) on 8 TRN2 NeuronCores — v2.

Math (per reference):
    agg[r] = segment_sum(vals[r][:,None] * inp[src[r]], dst[r], N)   # [N, F]
    out    = sum_r agg[r] @ W[r]                                     # [N, F]

Strategy: shard OUTPUT NODES (dst) across the 8 cores (6250 nodes each) —
no collective needed. The random-row dma_gather stream is the hard
bottleneck (~5 ns/row/core, transaction-limited), so v2 minimizes gathered
rows and hides everything else under that stream:

  * Edges are grouped by (dst-tile of 128 nodes, src-half) only — NOT per
    relation — and sorted by relation within the group. Chunk padding is
    per (tile, half) instead of per (tile, relation, half): ~110k gathered
    rows/core vs ~150k for per-relation chunking.
  * Gathers go to 4 SWDGE queues round-robin (deeper DMA pipelining,
    ~1.6x on the gather stream).
  * Edges are sorted by src within each relation run (order inside a
    group is free) so the DMA descriptor stream walks ascending DRAM
    addresses.
  * inp is host-cast to fp16: 256B rows (same transaction cost, half SBUF).
  * Aggregation via "segment matmul" into a PSUM strip [128 fi, 8*128]
    (all relations of one dst tile): each chunk's S matrix [128 lanes,
    512 cols] covers a 4-relation window at (r - base)*128 + dstloc.
    S is built ON-CHIP by one fused DVE tensor_scalar per chunk:
    S = (iota == ctgt) * val with per-partition scalar operands. No
    dense S matrix ever touches HBM.
  * out_tile[n, fo] = sum_r aggT[:, r-slice].T @ W[r], all fp16 matmuls,
    PSUM->SBUF copies on the (otherwise idle) Activation engine.

SPMD: one program for all 8 cores. The chunk grid (chunks per (tile,
half), per-chunk 4-relation window base) is shared across cores (max/union
over cores); per-core data (gather indices, target columns, values) pads
with idx 0 / column -1000 / value 0, contributing nothing.
"""

import numpy as np

# Problem constants (hardcoded per harness contract).
N, R, E, F = 50000, 8, 100000, 128
C = 8                      # cores
NPC = N // C               # 6250 dst nodes per core
TILE = 128                 # dst nodes per tile
T = -(-NPC // TILE)        # 49 tiles per core (last has 106 nodes)
HALF = 32768               # int16 gather-index boundary
W_WIN = 512                # S window: 4 relations * 128 dst cols
PAD_COL = -1000            # ctgt value for pad lanes (never matches iota)

F32 = np.float32
FP16 = np.float16


def _build_layout(src, dst, vals):
    """Shared chunk grid + per-core device arrays.

    Returns (meta, per_core): meta has the shared program structure,
    per_core[c] the input arrays for core c.
    """
    src = np.asarray(src)
    dst = np.asarray(dst)
    vals = np.asarray(vals)

    # per[c][t][sec] -> list over r of (r, dstloc, src_adj, val)
    per = [[[[], []] for _ in range(T)] for _ in range(C)]
    for r in range(R):
        d = dst[r]
        s = src[r]
        v = vals[r]
        order = np.argsort(d, kind="stable")
        ds = d[order]
        for c in range(C):
            a = np.searchsorted(ds, c * NPC, "left")
            b = np.searchsorted(ds, (c + 1) * NPC, "left")
            eidx = order[a:b]
            dl = ds[a:b] - c * NPC
            for t in range(T):
                ta = np.searchsorted(dl, t * TILE, "left")
                tb = np.searchsorted(dl, (t + 1) * TILE, "left")
                ge = eidx[ta:tb]
                gs = s[ge]
                gd = dl[ta:tb] - t * TILE
                gv = v[ge]
                lo = gs < HALF
                for sec, m, sub in ((0, lo, 0), (1, ~lo, HALF)):
                    gdm, gsm, gvm = gd[m], gs[m] - sub, gv[m]
                    o = np.argsort(gsm, kind="stable")
                    per[c][t][sec].append((r, gdm[o], gsm[o], gvm[o]))

    # Per-core flattened (r-major) streams per (t, sec).
    # stream[c][t][sec] = (rvec, dvec, svec, vvec)
    stream = [[[None, None] for _ in range(T)] for _ in range(C)]
    for c in range(C):
        for t in range(T):
            for sec in range(2):
                parts = per[c][t][sec]
                rv = np.concatenate(
                    [np.full(len(p[1]), p[0], np.int16) for p in parts])
                dv = np.concatenate([p[1] for p in parts]).astype(np.int16)
                sv = np.concatenate([p[2] for p in parts]).astype(np.int16)
                vv = np.concatenate([p[3] for p in parts]).astype(FP16)
                stream[c][t][sec] = (rv, dv, sv, vv)

    # Shared chunk grid: nch per (t, sec) = max over cores; per-chunk
    # 4-relation window base from the union of relations present.
    tiles = []          # per t: dict(nch=[lo,hi], bases=[...], offs...)
    nct = 0             # cumulative chunk count (ctgt/cval columns)
    gcols = [0, 0]      # cumulative gidx cols per sec
    cap = 0
    for t in range(T):
        info = dict(nch=[0, 0], bases=[[], []], coff=nct, goff=[0, 0])
        for sec in range(2):
            ncs = [len(stream[c][t][sec][0]) for c in range(C)]
            nch = max(-(-n // 128) for n in ncs)
            info["nch"][sec] = nch
            info["goff"][sec] = gcols[sec]
            bases = []
            for ch in range(nch):
                rmin, rmax = R, -1
                for c in range(C):
                    rv = stream[c][t][sec][0][ch * 128:(ch + 1) * 128]
                    if len(rv):
                        rmin = min(rmin, int(rv[0]))
                        rmax = max(rmax, int(rv[-1]))
                assert rmax >= 0
                base = min(rmin, R - W_WIN // 128)
                assert rmax - base < W_WIN // 128, (
                    f"chunk spans >{W_WIN // 128} relations: t={t} sec={sec} "
                    f"ch={ch} rmin={rmin} rmax={rmax}")
                bases.append(base)
            info["bases"][sec] = bases
            gcols[sec] += nch * 8
            nct += nch
        cap = max(cap, info["nch"][0] + info["nch"][1])
        tiles.append(info)

    meta = dict(tiles=tiles, nct=nct, cap=cap,
                lo_cols=gcols[0], hi_cols=gcols[1])

    # Per-core data arrays.
    per_core = []
    for c in range(C):
        gidx = [np.zeros((16, max(gcols[0], 8)), np.int16),
                np.zeros((16, max(gcols[1], 8)), np.int16)]
        ctgt = np.full((128, nct), PAD_COL, F32)
        cval = np.zeros((128, nct), F32)
        for t in range(T):
            info = tiles[t]
            coff = info["coff"]
            for sec in range(2):
                nch = info["nch"][sec]
                if nch == 0:
                    continue
                rv, dv, sv, vv = stream[c][t][sec]
                ne = len(rv)
                vec = np.zeros(nch * 128, np.int16)
                vec[:ne] = sv
                goff = info["goff"][sec]
                gidx[sec][:, goff:goff + nch * 8] = vec.reshape(-1, 16).T
                i = np.arange(ne)
                ch = i // 128
                lane = i % 128
                bases = np.asarray(info["bases"][sec], np.int32)
                col = (rv.astype(np.int32) - bases[ch]) * 128 \
                    + dv.astype(np.int32)
                assert col.min() >= 0 and col.max() < W_WIN
                cc = coff + (0 if sec == 0 else info["nch"][0]) + ch
                ctgt[lane, cc] = col.astype(F32)
                cval[lane, cc] = vv
            coff += info["nch"][0] + info["nch"][1]
        per_core.append(dict(
            gidx_lo=np.tile(gidx[0], (8, 1)),
            gidx_hi=np.tile(gidx[1], (8, 1)),
            ctgt=ctgt,
            cval=cval,
        ))
    return meta, per_core


def _build_program(meta, reps=1):
    import concourse.bacc as bacc
    import concourse.mybir as mybir
    import concourse.tile as tile

    f32 = mybir.dt.float32
    fp16 = mybir.dt.float16
    i16 = mybir.dt.int16
    eq = mybir.AluOpType.is_equal
    mult = mybir.AluOpType.mult

    nc = bacc.Bacc(None, target_bir_lowering=False, num_swdge_queues=4)

    nct = meta["nct"]
    cap = meta["cap"]
    inp_d = nc.dram_tensor("inp", [N, F], fp16, kind="ExternalInput")
    gilo_d = nc.dram_tensor("gidx_lo", [128, max(meta["lo_cols"], 8)], i16,
                            kind="ExternalInput")
    gihi_d = nc.dram_tensor("gidx_hi", [128, max(meta["hi_cols"], 8)], i16,
                            kind="ExternalInput")
    ctgt_d = nc.dram_tensor("ctgt", [128, nct], f32, kind="ExternalInput")
    cval_d = nc.dram_tensor("cval", [128, nct], f32, kind="ExternalInput")
    iota_d = nc.dram_tensor("iota", [128, W_WIN], fp16, kind="ExternalInput")
    w_d = nc.dram_tensor("weights", [128, R * F], fp16, kind="ExternalInput")
    out_d = nc.dram_tensor("out", [NPC, F], f32, kind="ExternalOutput")

    with tile.TileContext(nc) as tc:
        with (
            tc.tile_pool(name="const", bufs=1) as cpool,
            tc.tile_pool(name="gbuf", bufs=6) as gpool,
            tc.tile_pool(name="stile", bufs=4) as spool,
            tc.tile_pool(name="aggsb", bufs=3) as apool,
            tc.tile_pool(name="osb", bufs=3) as opool,
            tc.tile_pool(name="psA", bufs=2, space="PSUM") as psum_a,
            tc.tile_pool(name="psO", bufs=2, space="PSUM") as psum_o,
        ):
            wtile = cpool.tile([128, R * F], fp16)
            nc.sync.dma_start(wtile[:], w_d[:])
            gilo = cpool.tile([128, max(meta["lo_cols"], 8)], i16)
            nc.sync.dma_start(gilo[:], gilo_d[:])
            gihi = cpool.tile([128, max(meta["hi_cols"], 8)], i16)
            nc.sync.dma_start(gihi[:], gihi_d[:])
            ctgt = cpool.tile([128, nct], f32)
            nc.sync.dma_start(ctgt[:], ctgt_d[:])
            cval = cpool.tile([128, nct], f32)
            nc.sync.dma_start(cval[:], cval_d[:])
            iota = cpool.tile([128, W_WIN], fp16)
            nc.sync.dma_start(iota[:], iota_d[:])
            szero = cpool.tile([128, W_WIN], fp16)
            nc.gpsimd.memset(szero[:], 0.0)

            qn = 0
            for _rep in range(reps):
                for t in range(T):
                    info = meta["tiles"][t]
                    nlo, nhi = info["nch"]
                    ntot = nlo + nhi
                    coff = info["coff"]

                    gbuf = gpool.tile([128, cap, F], fp16, tag="gbuf")
                    for (sec, nch, off, gi_t, src_ap) in (
                        (0, nlo, 0, gilo, inp_d[0:HALF, :]),
                        (1, nhi, nlo, gihi, inp_d[HALF:N, :]),
                    ):
                        goff = info["goff"][sec]
                        for c0 in range(0, nch, 8):
                            cn = min(8, nch - c0)
                            nc.gpsimd.dma_gather(
                                gbuf[:, off + c0: off + c0 + cn, :], src_ap,
                                gi_t[:, goff + c0 * 8: goff + (c0 + cn) * 8],
                                cn * 128, cn * 128, F,
                                queue_num=qn % 4,
                            )
                            qn += 1

                    # S on-chip: stile[l, ch, col] = (col == ctgt[l,ch]) * val
                    # — one fused DVE tensor_scalar per chunk, per-partition
                    # scalars (exempt from the 2x-mode packing rules).
                    stile = spool.tile([128, cap, W_WIN], fp16, tag="stile")
                    for ch in range(ntot):
                        cc = coff + ch
                        nc.vector.tensor_scalar(
                            stile[:, ch, :], iota[:],
                            ctgt[:, cc:cc + 1], cval[:, cc:cc + 1],
                            eq, mult)

                    # aggregation strip [fi, r*128 + dstloc]
                    agg = psum_a.tile([128, R * F], f32, tag="agg")
                    for h in range(2):
                        nc.tensor.matmul(
                            agg[:, h * W_WIN:(h + 1) * W_WIN],
                            wtile[:, 0:128], szero[:],
                            start=True, stop=False, skip_group_check=True)
                    for ch in range(ntot):
                        sec = 0 if ch < nlo else 1
                        base = info["bases"][sec][ch - (nlo if sec else 0)]
                        o = base * 128
                        nc.tensor.matmul(
                            agg[:, o:o + W_WIN],
                            gbuf[:, ch, :], stile[:, ch, :],
                            start=False, stop=(ch == ntot - 1),
                            skip_group_check=True)

                    aggsb = apool.tile([128, R * F], fp16, tag="aggsb")
                    nc.scalar.copy(aggsb[:], agg[:])

                    out_ps = psum_o.tile([128, F], f32, tag="ops")
                    for r in range(R):
                        nc.tensor.matmul(
                            out_ps[:],
                            aggsb[:, r * F:(r + 1) * F],
                            wtile[:, r * F:(r + 1) * F],
                            start=(r == 0), stop=(r == R - 1))
                    osb = opool.tile([128, F], f32, tag="osb")
                    nc.scalar.copy(osb[:], out_ps[:])
                    rows = min(TILE, NPC - t * TILE)
                    nc.sync.dma_start(
                        out_d[t * TILE: t * TILE + rows, :], osb[0:rows, :])

    nc.compile()
    return nc


def _make_in_map(inp, weights, pc):
    weights = np.asarray(weights, F32)
    wflat = np.zeros((128, R * F), FP16)
    for r in range(R):
        wflat[:, r * F:(r + 1) * F] = weights[r].astype(FP16)
    iota = np.tile(np.arange(W_WIN, dtype=FP16).reshape(1, W_WIN), (128, 1))
    return dict(inp=np.asarray(inp).astype(FP16), weights=wflat,
                gidx_lo=pc["gidx_lo"], gidx_hi=pc["gidx_hi"],
                ctgt=pc["ctgt"], cval=pc["cval"], iota=iota)


def kernel(inp, src, dst, vals, weights):
    from concourse.bass_utils import run_bass_kernel_spmd

    inp = np.asarray(inp, F32)
    weights = np.asarray(weights, F32)
    meta, per_core = _build_layout(src, dst, vals)
    nc = _build_program(meta)

    in_maps = [_make_in_map(inp, weights, pc) for pc in per_core]
    res = run_bass_kernel_spmd(nc, in_maps, core_ids=list(range(C)))
    out = np.concatenate([res.results[c]["out"] for c in range(C)], axis=0)
    return out.astype(F32)
